# revision 1
# baseline (speedup 1.0000x reference)
"""Linear-chain CRF forward (log partition) on 8 Trainium2 NeuronCores.

Strategy (data-parallel over batch, 16 rows/core):
  The log-space recursion
      alpha_t[b,to] = feats[b,t,to] + LSE_from(alpha_{t-1}[b,from] + trans[from,to])
  is run in exp space:
      A_t = (A_{t-1} @ W') * E_t,   W' = exp(trans - C),  E_t = exp(feats_t)
  with A kept *transposed* on chip as [to (2x128 partitions), b (16 free)], so
  each step is 4 small matmuls (stationary W' chunks, moving A) whose PSUM
  output is already in the layout the next step consumes -- no transposes.
  Every `nr` steps a per-row scale r = 1/sum(A) is folded into the next E tile
  (off the critical path) and logged; logZ = log(z) - sum(log r) + n_mm*C.

  alpha_0 is seeded by running the same step with A_{-1} = one-hot(START);
  the final transition to STOP is one more matmul (the STOP column of W').
"""

import numpy as np

import concourse.bacc as bacc
import concourse.bass as bass
import concourse.mybir as mybir
import concourse.tile as tile
from concourse.bass_utils import run_bass_kernel_spmd

F32 = mybir.dt.float32
BF16 = mybir.dt.bfloat16
AF = mybir.ActivationFunctionType

B, T, G = 128, 512, 256
NCORES = 8
BC = B // NCORES          # batch rows per core
START, STOP = G - 2, G - 1
C = 6.0                   # per-matmul constant log-scale folded into W'
NR = 64                   # renorm cadence (steps)
TB = 64                   # feats time-block size
NB = T // TB
N_MM = T + 1              # matmuls that carry the e^-C factor

# config used by kernel() -- the best measured variant
BEST = dict(nr=NR, merged=False, ps_bufs=2, a_bufs=3)

_CACHE: dict = {}


def _build_program(repeat: int = 1, nr: int = NR, merged: bool = False,
                   ps_bufs: int = 2, a_bufs: int = 3, chains: int = 0,
                   probe: str | None = None,
                   palindrome: bool = False) -> bass.Bass:
    """repeat>1 re-runs the whole E-pipeline + recursion (timing only).

    chains>0 selects the multi-chain structure: the 16 batch rows split
    into `chains` independent recursions so their serial latencies hide
    under each other. probe="dma" drops the recursion (DMA/exp pipeline
    timing only; output is garbage).
    """
    renorm_ts = set(t for t in range(T) if (t + 1) % nr == 0 and (t + 1) < T)
    n_renorm = len(renorm_ts)
    nc = bacc.Bacc("TRN2", target_bir_lowering=False, debug=False,
                   num_devices=NCORES)
    featsT = nc.dram_tensor("featsT", [128, T, 2, BC], F32, kind="ExternalInput")
    trans = nc.dram_tensor("trans", [G, G], F32, kind="ExternalInput")
    logz = nc.dram_tensor("logz", [1, BC], F32, kind="ExternalOutput")

    with tile.TileContext(nc) as tc:
        with (
            tc.tile_pool(name="wpool", bufs=1) as wpool,
            tc.tile_pool(name="stage", bufs=2) as stage_pool,
            tc.tile_pool(name="epool", bufs=3) as e_pool,
            tc.tile_pool(name="apool", bufs=a_bufs) as a_pool,
            tc.tile_pool(name="escp", bufs=2) as esc_pool,
            tc.tile_pool(name="misc", bufs=1) as misc,
            tc.tile_pool(name="ps0", bufs=ps_bufs, space="PSUM") as ps0_pool,
            tc.tile_pool(name="ps1", bufs=ps_bufs, space="PSUM") as ps1_pool,
            tc.tile_pool(name="pss", bufs=1, space="PSUM") as pss_pool,
        ):
            # ---- weights: W'[from,to] = exp(trans - C), as 2 from-chunk
            # tiles, padded with a [STOP-col, 0...] block so the final mm's
            # stationary load is a full 128 columns (LDW-opt compatible).
            biasC = wpool.tile([128, 1], F32, name="biasC")
            nc.vector.memset(biasC[:], -C)
            wk = []
            for k in range(2):
                wt = wpool.tile([128, G], F32, name=f"wt{k}")
                nc.sync.dma_start(wt[:], trans[k * 128:(k + 1) * 128, :])
                wb = wpool.tile([128, G + 128], BF16, name=f"wb{k}")
                nc.vector.memset(wb[:, G:G + 128], 0.0)
                nc.scalar.activation(wb[:, 0:G], wt[:], AF.Exp, bias=biasC[:])
                nc.vector.tensor_copy(wb[:, G:G + 1], wb[:, STOP:STOP + 1])
                wk.append(wb)

            ones128 = wpool.tile([128, 128], BF16, name="ones128")
            nc.vector.memset(ones128[:], 1.0)
            ones_row = wpool.tile([1, 128], BF16, name="ones_row")
            nc.vector.memset(ones_row[:], 1.0)

            rbuf = misc.tile([1, max(n_renorm, 1) * BC], F32, name="rbuf")

            def emit_renorm(rep, t, a_chunk0, a_chunk1, eblocks, ri):
                """s[b]=sum_to A_t bcast over partitions; r=1/s logged,
                folded into E_{t+1}."""
                s_ps = pss_pool.tile([128, BC], F32, name=f"s_{rep}_{t}",
                                     tag="s")
                nc.tensor.matmul(s_ps[:], ones128[:], a_chunk0,
                                 start=True, stop=False)
                nc.tensor.matmul(s_ps[:], ones128[:], a_chunk1,
                                 start=False, stop=True)
                r2 = esc_pool.tile([128, 2 * BC], F32,
                                   name=f"r2_{rep}_{t}", tag="rsc")
                nc.vector.reciprocal(r2[:, 0:BC], s_ps[:])
                nc.vector.reciprocal(r2[:, BC:2 * BC], s_ps[:])
                # record the *applied* (fp32) scale exactly
                nc.vector.tensor_copy(rbuf[:, ri * BC:(ri + 1) * BC],
                                      r2[0:1, 0:BC])
                ebn = eblocks[(t + 1) // TB]
                offn = ((t + 1) % TB) * 2 * BC
                esc = esc_pool.tile([128, 2 * BC], F32,
                                    name=f"esc{rep}_{t}", tag="esc")
                nc.vector.tensor_mul(esc[:], ebn[:, offn:offn + 2 * BC],
                                     r2[:])
                return esc

            def emit_epipe(rep: int):
                eblocks = []
                for blk in range(NB):
                    st = stage_pool.tile([128, TB * 2 * BC], F32,
                                         name=f"st{rep}_{blk}", tag="st")
                    src = featsT[:, blk * TB:(blk + 1) * TB, :, :]
                    nc.sync.dma_start(st[:],
                                      src.rearrange("p t c b -> p (t c b)"))
                    eb = e_pool.tile([128, TB * 2 * BC], F32,
                                     name=f"eb{rep}_{blk}", tag="eb")
                    nc.scalar.activation(eb[:], st[:], AF.Exp)
                    eblocks.append(eb)
                return eblocks

            def one_pass(rep: int):
                """E-pipeline + full recursion; returns final A chunk APs."""
                eblocks = emit_epipe(rep)

                # A_{-1} = one-hot(START) over [to, b]
                if merged:
                    ap = a_pool.tile([128, 2 * BC], BF16,
                                     name=f"ai{rep}", tag="a")
                    nc.vector.memset(ap[:], 0.0)
                    nc.sync.dma_start(ap[START - 128:START - 127, BC:2 * BC],
                                      ones_row[0:1, 0:BC])
                    a0p, a1p = ap[:, 0:BC], ap[:, BC:2 * BC]
                else:
                    a0t = a_pool.tile([128, BC], BF16,
                                      name=f"a0i{rep}", tag="a0")
                    nc.vector.memset(a0t[:], 0.0)
                    a1t = a_pool.tile([128, BC], BF16,
                                      name=f"a1i{rep}", tag="a1")
                    nc.vector.memset(a1t[:], 0.0)
                    nc.sync.dma_start(a1t[START - 128:START - 127, :],
                                      ones_row[0:1, 0:BC])
                    a0p, a1p = a0t[:], a1t[:]

                esc_pending = None  # scaled E tile for the upcoming step
                ri = 0
                for t in range(T):
                    if esc_pending is not None:
                        e0 = esc_pending[:, 0:BC]
                        e1 = esc_pending[:, BC:2 * BC]
                        e01 = esc_pending[:, 0:2 * BC]
                        esc_pending = None
                    else:
                        eb = eblocks[t // TB]
                        off = (t % TB) * 2 * BC
                        e0 = eb[:, off:off + BC]
                        e1 = eb[:, off + BC:off + 2 * BC]
                        e01 = eb[:, off:off + 2 * BC]

                    if merged:
                        ps = ps0_pool.tile([128, 2 * BC], F32,
                                           name=f"ps_{rep}_{t}", tag="p0")
                        rhs = {0: a0p, 1: a1p}
                        # (k, m) order; odd steps reversed so identical
                        # weight chunks abut across step boundaries and the
                        # walrus LDW-elision can drop the reload
                        order = [(0, 0), (1, 0), (0, 1), (1, 1)]
                        if palindrome and (t % 2 == 1):
                            order = order[::-1]
                        seen_m = set()
                        for k, m in order:
                            nc.tensor.matmul(
                                ps[:, m * BC:(m + 1) * BC],
                                wk[k][:, m * 128:(m + 1) * 128], rhs[k],
                                start=m not in seen_m,
                                stop=m in seen_m)
                            seen_m.add(m)
                        an = a_pool.tile([128, 2 * BC], BF16,
                                         name=f"a_{rep}_{t}", tag="a")
                        nc.vector.tensor_mul(an[:], ps[:], e01)
                        a0p, a1p = an[:, 0:BC], an[:, BC:2 * BC]
                    else:
                        ps0 = ps0_pool.tile([128, BC], F32,
                                            name=f"ps0_{rep}_{t}", tag="p0")
                        nc.tensor.matmul(ps0[:], wk[0][:, 0:128], a0p,
                                         start=True, stop=False)
                        nc.tensor.matmul(ps0[:], wk[1][:, 0:128], a1p,
                                         start=False, stop=True)
                        a0 = a_pool.tile([128, BC], BF16,
                                         name=f"a0_{rep}_{t}", tag="a0")
                        nc.vector.tensor_mul(a0[:], ps0[:], e0)

                        ps1 = ps1_pool.tile([128, BC], F32,
                                            name=f"ps1_{rep}_{t}", tag="p1")
                        nc.tensor.matmul(ps1[:], wk[0][:, 128:256], a0p,
                                         start=True, stop=False)
                        nc.tensor.matmul(ps1[:], wk[1][:, 128:256], a1p,
                                         start=False, stop=True)
                        a1 = a_pool.tile([128, BC], BF16,
                                         name=f"a1_{rep}_{t}", tag="a1")
                        nc.vector.tensor_mul(a1[:], ps1[:], e1)
                        a0p, a1p = a0[:], a1[:]

                    if t in renorm_ts:
                        esc_pending = emit_renorm(rep, t, a0p, a1p,
                                                  eblocks, ri)
                        ri += 1
                return a0p, a1p

            def cview(ap2d):
                """[p, 2*n] flat AP -> [p, 2, n] (chunk-major) view."""
                return ap2d.rearrange("p (c b) -> p c b", c=2)

            def one_pass_chains(rep: int):
                """`chains` independent recursions over disjoint b-ranges."""
                bcn = BC // chains
                eblocks = (emit_epipe(rep)
                           if probe not in ("pe4", "pe2") else [])
                aps = []
                for g in range(chains):
                    at = a_pool.tile([128, 2 * bcn], BF16,
                                     name=f"ai{rep}_{g}", tag=f"a{g}")
                    nc.vector.memset(at[:], 0.0)
                    nc.sync.dma_start(at[START - 128:START - 127, bcn:2 * bcn],
                                      ones_row[0:1, 0:bcn])
                    aps.append(at[:])
                if probe == "dma":
                    nc.vector.memset(rbuf[:], 1.0)
                    sc = nc.dram_tensor(f"probe_sc{rep}", [128, 1], F32)
                    for eb in eblocks:
                        nc.sync.dma_start(sc[:, :], eb[:, 0:1])
                    return aps
                if probe in ("pe4", "pe2"):
                    # pure PE throughput: 4 (or 2) matmuls/step off a fixed
                    # rhs, no DVE in the loop
                    nc.vector.memset(rbuf[:], 1.0)
                    nmm = 4 if probe == "pe4" else 2
                    a0 = aps[0]
                    for t in range(T):
                        ps = ps0_pool.tile([128, 2 * BC], F32,
                                           name=f"pp_{rep}_{t}", tag="p0")
                        for j in range(nmm):
                            k, m = j % 2, j // 2
                            nc.tensor.matmul(
                                ps[:, m * BC:(m + 1) * BC],
                                wk[k][:, m * 128:(m + 1) * 128],
                                a0[:, 0:BC],
                                start=(k == 0), stop=(k == 1))
                        last_ps = ps
                    dump = misc.tile([128, 2 * BC], F32, name=f"dump{rep}")
                    nc.vector.tensor_copy(dump[:], last_ps[:])
                    return aps

                esc_pending = [None] * chains
                ri = 0
                for t in range(T):
                    for g in range(chains):
                        if esc_pending[g] is not None:
                            ev = cview(esc_pending[g][:, 0:2 * bcn])
                            esc_pending[g] = None
                        else:
                            eb = eblocks[t // TB]
                            base = (t % TB) * 2 * BC
                            ev = cview(eb[:, base:base + 2 * BC])[
                                :, :, g * bcn:(g + 1) * bcn]
                        ap_prev = aps[g]
                        ps = ps0_pool.tile([128, 2 * bcn], F32,
                                           name=f"ps_{rep}_{t}_{g}",
                                           tag=f"p{g}")
                        nc.tensor.matmul(ps[:, 0:bcn], wk[0][:, 0:128],
                                         ap_prev[:, 0:bcn],
                                         start=True, stop=False)
                        nc.tensor.matmul(ps[:, 0:bcn], wk[1][:, 0:128],
                                         ap_prev[:, bcn:2 * bcn],
                                         start=False, stop=True)
                        nc.tensor.matmul(ps[:, bcn:2 * bcn],
                                         wk[0][:, 128:256],
                                         ap_prev[:, 0:bcn],
                                         start=True, stop=False)
                        nc.tensor.matmul(ps[:, bcn:2 * bcn],
                                         wk[1][:, 128:256],
                                         ap_prev[:, bcn:2 * bcn],
                                         start=False, stop=True)
                        an = a_pool.tile([128, 2 * bcn], BF16,
                                         name=f"a_{rep}_{t}_{g}", tag=f"a{g}")
                        nc.vector.tensor_mul(cview(an[:]), cview(ps[:]), ev)
                        aps[g] = an[:]

                    if t in renorm_ts:
                        s_ps = pss_pool.tile([128, BC], F32,
                                             name=f"s_{rep}_{t}", tag="s")
                        ebn = eblocks[(t + 1) // TB]
                        basen = ((t + 1) % TB) * 2 * BC
                        for g in range(chains):
                            sl = s_ps[:, g * bcn:(g + 1) * bcn]
                            nc.tensor.matmul(sl, ones128[:],
                                             aps[g][:, 0:bcn],
                                             start=True, stop=False)
                            nc.tensor.matmul(sl, ones128[:],
                                             aps[g][:, bcn:2 * bcn],
                                             start=False, stop=True)
                            r2 = esc_pool.tile([128, 2 * bcn], F32,
                                               name=f"r2_{rep}_{t}_{g}",
                                               tag=f"rsc{g}")
                            nc.vector.reciprocal(r2[:, 0:bcn], sl)
                            nc.vector.reciprocal(r2[:, bcn:2 * bcn], sl)
                            nc.vector.tensor_copy(
                                rbuf[:, ri * BC + g * bcn:
                                     ri * BC + (g + 1) * bcn],
                                r2[0:1, 0:bcn])
                            evn = cview(ebn[:, basen:basen + 2 * BC])[
                                :, :, g * bcn:(g + 1) * bcn]
                            esc = esc_pool.tile([128, 2 * bcn], F32,
                                                name=f"esc{rep}_{t}_{g}",
                                                tag=f"esc{g}")
                            nc.vector.tensor_mul(cview(esc[:]), evn,
                                                 cview(r2[:]))
                            esc_pending[g] = esc
                        ri += 1
                return aps

            if chains > 0:
                assert BC % chains == 0
                for rep in range(repeat):
                    aps = one_pass_chains(rep)
            else:
                assert probe is None
                for rep in range(repeat):
                    a0p, a1p = one_pass(rep)

            # ---- final: transition to STOP = one more matmul with the
            # padded [STOP-col, 0...] weight block (z lands at partition 0)
            zf = pss_pool.tile([128, BC], F32, name="zf", tag="zf")
            if chains > 0:
                bcn = BC // chains
                for g in range(chains):
                    sl = zf[:, g * bcn:(g + 1) * bcn]
                    nc.tensor.matmul(sl, wk[0][:, G:G + 128],
                                     aps[g][:, 0:bcn], start=True, stop=False)
                    nc.tensor.matmul(sl, wk[1][:, G:G + 128],
                                     aps[g][:, bcn:2 * bcn],
                                     start=False, stop=True)
            else:
                nc.tensor.matmul(zf[:], wk[0][:, G:G + 128], a0p,
                                 start=True, stop=False)
                nc.tensor.matmul(zf[:], wk[1][:, G:G + 128], a1p,
                                 start=False, stop=True)
            logq = misc.tile([1, BC], F32, name="logq")
            nc.scalar.activation(logq[:], zf[0:1, :], AF.Ln)
            rlog = misc.tile([1, max(n_renorm, 1) * BC], F32, name="rlog")
            nc.scalar.activation(rlog[:], rbuf[:], AF.Ln)
            slr = misc.tile([1, BC], F32, name="slr")
            nc.vector.tensor_reduce(
                slr[:],
                rlog[0:1, :].rearrange("p (k b) -> p b k", b=BC),
                axis=mybir.AxisListType.X,
                op=mybir.AluOpType.add,
            )
            lz0 = misc.tile([1, BC], F32, name="lz0")
            nc.vector.tensor_sub(lz0[:], logq[:], slr[:])
            lz1 = misc.tile([1, BC], F32, name="lz1")
            nc.vector.tensor_scalar_add(lz1[:], lz0[:], float(N_MM * C))
            nc.sync.dma_start(logz[:, :], lz1[:])

    nc.compile()
    return nc


def _marshal_inputs(feats: np.ndarray, transitions: np.ndarray):
    """Per-core input dicts. feats -> [to%128, t, to//128, b] fp32."""
    trans = np.ascontiguousarray(transitions, dtype=np.float32)
    in_maps = []
    for c in range(NCORES):
        fc = feats[c * BC:(c + 1) * BC]              # [BC, T, G]
        ft = fc.transpose(2, 1, 0)                   # [G, T, BC]
        ft = ft.reshape(2, 128, T, BC).transpose(1, 2, 0, 3)  # [128,T,2,BC]
        in_maps.append({
            "featsT": np.ascontiguousarray(ft, dtype=np.float32),
            "trans": trans,
        })
    return in_maps


def _get_program(repeat: int = 1, **cfg) -> bass.Bass:
    params = dict(BEST)
    params.update(cfg)
    key = ("nc", repeat, tuple(sorted(params.items())))
    if key not in _CACHE:
        _CACHE[key] = _build_program(repeat, **params)
    return _CACHE[key]


def _run(feats, transitions, trace=False, repeat=1, cfg=None, **spmd_kwargs):
    nc = _get_program(repeat, **(cfg or {}))
    in_maps = _marshal_inputs(np.asarray(feats), np.asarray(transitions))
    res = run_bass_kernel_spmd(nc, in_maps, list(range(NCORES)),
                               trace=trace, **spmd_kwargs)
    total = np.float64(0.0)
    for r in res.results:
        total += np.asarray(r["logz"], dtype=np.float64).sum()
    return np.float32(total), res


def kernel(feats: np.ndarray, mask: np.ndarray, transitions: np.ndarray) -> np.ndarray:
    assert bool(np.all(mask)), "kernel assumes an all-ones mask"
    out, _ = _run(feats, transitions, trace=False)
    return np.asarray(out, dtype=np.float32)

